# revision 35
# baseline (speedup 1.0000x reference)
"""Trainium2 Bass kernel for a hypernetwork-generated per-case MLP.

Math (fp32 reference):
  h = silu(o @ Wc + bc)                          [C=64, H=256]
  w = einsum('ch,lhd->lcd', h, Ww) + bw          [L=4, C, 65536]
  b = einsum('ch,lhd->lcd', h, Wb) + bb          [L=4, C, 256]
  per-case 4-layer MLP over shared x [2048, 256] with silu + skip:
    a0 = silu(x @ W0 + b0); a1 = silu(a0 @ W1 + b1)
    a2 = silu(a1 @ W2 + b2); out = (a2 + a0) @ W3 + b3      -> [C*N, 256]

Distribution over 8 NeuronCores (all matmuls fp16, full PE rate):
  - weight-gen tensor-sharded over the d axis of Ww: each core owns a
    contiguous 8192-wide shard and computes w[l, all 64 cases, shard]
    as [cases=64, d] matmuls (lhsT = hT, M=64);
  - one AllToAll per layer redistributes w so core k holds full-d
    weights for its 8 cases; the domain net is then data-parallel over
    cases with zero activation communication;
  - domain net runs layer-major in two groups of 4 cases so the
    per-layer AllToAll latency stays off the critical path; activations
    are kept feature-major [feat, n] so every layer is a plain
    lhsT=W[i,o], rhs=A[i,n] matmul with no transposes;
  - scheduling around two hardware facts: DMA queues are strict FIFO,
    and a collective trigger blocks its queue until the previous
    collective completes.  So the gpsimd queue carries only collective
    triggers, the 16MB Ww stream is split across the sync+scalar HW
    queues with layer 3 deferred to overlap the first AllToAll, and
    staging/weight/output DMAs are interleaved at matching readiness
    positions;
  - outputs written fp16 (cast to fp32 on host), halving the tail.
"""

import numpy as np

import concourse.bass as bass
import concourse.mybir as mybir
import concourse.tile as tile
from concourse import bacc
from concourse.bass import ts, ds
from concourse.bass_utils import run_bass_kernel_spmd

F32 = mybir.dt.float32
F16 = mybir.dt.float16
AF = mybir.ActivationFunctionType

P = 128
NCORES = 8
C = 64
CC = C // NCORES
CIN = 64
H = 256
HB = H // P
DIN = 256
IB = DIN // P
NL = 4
N = 2048
D = DIN * DIN
DSH = D // NCORES
QW = DSH // 4
GRP = 4
_nc_cache = {}


def _build():
    nc = bacc.Bacc("TRN2", target_bir_lowering=False, debug=False, num_devices=NCORES)

    xt = nc.dram_tensor("xt", [P, IB, N], F16, kind="ExternalInput").ap()
    ot = nc.dram_tensor("ot", [P, C], F16, kind="ExternalInput").ap()
    oto = nc.dram_tensor("oto", [P, CC], F16, kind="ExternalInput").ap()
    wc = nc.dram_tensor("wc", [P, H], F16, kind="ExternalInput").ap()
    bc2 = nc.dram_tensor("bc2", [P, HB], F32, kind="ExternalInput").ap()
    wws = nc.dram_tensor("wws", [NL, H, DSH], F16, kind="ExternalInput").ap()
    wbT = nc.dram_tensor("wbT", [P, HB, NL, DIN], F16, kind="ExternalInput").ap()
    bbT = nc.dram_tensor("bbT", [P, IB, NL], F32, kind="ExternalInput").ap()
    bwT = nc.dram_tensor("bwT", [P, NL, IB, DIN], F16, kind="ExternalInput").ap()
    yt = nc.dram_tensor("yt", [CC, IB, P, N], F16, kind="ExternalOutput").ap()

    with tile.TileContext(nc) as tc:
        with (
            tc.tile_pool(name="const", bufs=1) as const,
            tc.tile_pool(name="dram", bufs=1, space="DRAM") as dram,
            tc.tile_pool(name="ww", bufs=4) as ww,
            tc.tile_pool(name="wstg", bufs=5) as wstg,
            tc.tile_pool(name="wt", bufs=12) as wtp,
            tc.tile_pool(name="act", bufs=12) as act,
            tc.tile_pool(name="ps", bufs=2, space="PSUM") as psp,
        ):
            wc_sb = const.tile([P, H], F16)
            nc.sync.dma_start(wc_sb[:], wc)
            bc_sb = const.tile([P, HB], F32)
            nc.sync.dma_start(bc_sb[:], bc2)
            ot_sb = const.tile([P, C], F16)
            nc.sync.dma_start(ot_sb[:], ot)
            oto_sb = const.tile([P, CC], F16)
            nc.sync.dma_start(oto_sb[:], oto)
            wbT_sb = const.tile([P, HB, NL, DIN], F16)
            nc.sync.dma_start(wbT_sb[:], wbT)
            bbT_sb = const.tile([P, IB, NL], F32)
            nc.sync.dma_start(bbT_sb[:], bbT)
            # wws tiles created in consumption order; loads emitted on a
            # schedule so staging writes interleave into the queues without
            # head-of-line blocking (sync: q0/q2, scalar: q1/q3)
            wws_tiles = [
                ww.tile([P, HB, QW], F16, tag="wwt", name=f"wwt_{l}_{q}")
                for l in range(NL)
                for q in range(4)
            ]
            wws_views = [
                wws[l].rearrange("(kb p) d -> p kb d", p=P) for l in range(NL)
            ]

            def emit_wws(l, engs=(0, 1, 0, 1)):
                for q in range(4):
                    eng = (nc.sync, nc.scalar, nc.gpsimd)[engs[q]]
                    eng.dma_start(
                        wws_tiles[l * 4 + q][:], wws_views[l][:, :, ts(q, QW)]
                    )

            emit_wws(0)
            emit_wws(1)
            xt_sb = const.tile([P, IB, N], F16)
            bwT_sb = const.tile([P, NL, IB, DIN], F16)

            hT_sb = const.tile([P, HB, C], F16)
            hTo_sb = const.tile([P, HB, CC], F16)
            for kb in range(HB):
                ps = psp.tile([P, 2048], F32, tag="ps", name=f"psh{kb}")
                nc.tensor.matmul(
                    ps[:, :C],
                    lhsT=wc_sb[:, ts(kb, P)],
                    rhs=ot_sb,
                    start=True,
                    stop=True,
                )
                nc.scalar.activation(hT_sb[:, kb, :], ps[:, :C], AF.Silu, bias=bc_sb[:, kb : kb + 1])
                ps2 = psp.tile([P, 2048], F32, tag="ps", name=f"psh2{kb}")
                nc.tensor.matmul(
                    ps2[:, :CC],
                    lhsT=wc_sb[:, ts(kb, P)],
                    rhs=oto_sb,
                    start=True,
                    stop=True,
                )
                nc.scalar.activation(hTo_sb[:, kb, :], ps2[:, :CC], AF.Silu, bias=bc_sb[:, kb : kb + 1])

            bO_sb = const.tile([P, IB, NL, CC], F32)
            for l in range(NL):
                for ob in range(IB):
                    ps = psp.tile([P, 2048], F32, tag="ps", name=f"psb{l}{ob}")
                    for kb in range(HB):
                        nc.tensor.matmul(
                            ps[:, :CC],
                            lhsT=wbT_sb[:, kb, l, ts(ob, P)],
                            rhs=hTo_sb[:, kb, :],
                            start=(kb == 0),
                            stop=(kb == HB - 1),
                        )
                    nc.scalar.activation(
                        bO_sb[:, ob, l, :], ps[:, :CC], AF.Identity, bias=bbT_sb[:, ob, l : l + 1]
                    )

            w_fulls = []
            for l in range(NL):
                w_shard = dram.tile([C, DSH], F16, name=f"w_shard{l}")
                w_full = dram.tile([C, DSH], F16, name=f"w_full{l}")
                w_fulls.append(w_full)
                stgs = []
                for q in range(4):
                    wwt = wws_tiles[l * 4 + q]
                    ps = psp.tile([P, 2048], F32, tag="ps", name=f"psw{l}{q}")
                    for ch in range(QW // 512):
                        for kb in range(HB):
                            nc.tensor.matmul(
                                ps[:C, ts(ch, 512)],
                                lhsT=hT_sb[:, kb, :],
                                rhs=wwt[:, kb, ts(ch, 512)],
                                start=(kb == 0),
                                stop=(kb == HB - 1),
                            )
                    stg = wstg.tile([C, QW], F16, tag="wstg")
                    nc.vector.tensor_copy(stg[:], ps[:C, :])
                    stgs.append(stg)
                for q in range(4):
                    nc.scalar.dma_start(w_shard[:, ts(q, QW)], stgs[q][:])
                nc.gpsimd.collective_compute(
                    "AllToAll",
                    mybir.AluOpType.bypass,
                    replica_groups=[list(range(NCORES))],
                    ins=[w_shard.opt()],
                    outs=[w_full.opt()],
                )
                if l == 0:
                    emit_wws(2)
                if l == 1:
                    nc.sync.dma_start(xt_sb[:], xt)
                if l == 2:
                    emit_wws(3, engs=(0, 0, 0, 0))
                    nc.scalar.dma_start(bwT_sb[:], bwT)

            wf_views = [wf.rearrange("(j c) (il o) -> j c il o", c=CC, o=DIN) for wf in w_fulls]

            def load_wt(l, c):
                wts = []
                for ib in range(IB):
                    wt_t = wtp.tile([P, DIN], F16, tag="wt")
                    eng = (nc.sync, nc.scalar)[ib]
                    eng.dma_start(wt_t[:], wf_views[l][ds(GRP * ib, GRP), c])
                    nc.vector.tensor_add(wt_t[:], wt_t[:], bwT_sb[:, l, ib, :])
                    wts.append(wt_t)
                return wts

            # layer 0 for ALL cases first: AllToAll-0 already delivered every
            # case's l0 weights, and 8 case-layers of l0 work (~34us) buys
            # slack for the rest of the AllToAll chain to land
            seq = [(0, c) for c in range(CC)]
            for g in range(CC // GRP):
                for l in range(1, NL):
                    for c in range(g * GRP, (g + 1) * GRP):
                        seq.append((l, c))

            a_cur = [None] * CC
            a_skip = [None] * CC
            wts_next = load_wt(*seq[0])
            for i, (l, c) in enumerate(seq):
                wts = wts_next
                if i + 1 < len(seq):
                    wts_next = load_wt(*seq[i + 1])
                if True:
                    if True:
                        a_prev = xt_sb if l == 0 else a_cur[c]
                        # group-B skip tiles outlive the main ring rotation;
                        # a separate tag keeps the pool free of WAR cycles
                        if l == 0 and c >= GRP:
                            a_new = act.tile(
                                [P, IB, N], F16, tag="actb", bufs=GRP, name=f"a_{c}_{l}"
                            )
                        else:
                            a_new = act.tile([P, IB, N], F16, tag="act", name=f"a_{c}_{l}")
                        func = AF.Silu if l < NL - 1 else AF.Identity
                        a_sum = (
                            act.tile([P, IB, N], F16, tag="act", name=f"asum_{c}")
                            if l == 2
                            else None
                        )
                        for ob in range(IB):
                            ps = psp.tile([P, 2048], F32, tag="ps", name=f"psd_{c}_{l}_{ob}")
                            for nchunk in range(4):
                                for ib in range(IB):
                                    nc.tensor.matmul(
                                        ps[:, ts(nchunk, 512)],
                                        lhsT=wts[ib][:, ts(ob, P)],
                                        rhs=a_prev[:, ib, ts(nchunk, 512)],
                                        start=(ib == 0),
                                        stop=(ib == IB - 1),
                                    )
                            nc.scalar.activation(
                                a_new[:, ob, :], ps, func, bias=bO_sb[:, ob, l, c : c + 1]
                            )
                            if l == 2:
                                nc.vector.tensor_add(
                                    a_sum[:, ob, :], a_new[:, ob, :], a_skip[c][:, ob, :]
                                )
                            if l == NL - 1:
                                (nc.sync, nc.scalar)[ob].dma_start(
                                    yt[c, ob], a_new[:, ob, :]
                                )
                        if l == 0:
                            a_skip[c] = a_new
                        a_cur[c] = a_sum if l == 2 else a_new

    nc.compile()
    return nc


def _prep_inputs(x, o, Wc, bc, Ww, bw, Wb, bb):
    x = np.asarray(x, np.float32)
    o = np.asarray(o, np.float32)
    Wc = np.asarray(Wc, np.float32)
    bc = np.asarray(bc, np.float32)
    Ww = np.asarray(Ww, np.float32)
    bw = np.asarray(bw, np.float32)
    Wb = np.asarray(Wb, np.float32)
    bb = np.asarray(bb, np.float32)

    xt = np.ascontiguousarray(x.T.reshape(IB, P, N).transpose(1, 0, 2)).astype(np.float16)
    otf = np.zeros((P, C), np.float16)
    otf[:CIN, :] = o.T
    wcp = np.zeros((P, H), np.float16)
    wcp[:CIN, :] = Wc
    bc2 = np.ascontiguousarray(bc.reshape(HB, P).T)
    wbT = np.ascontiguousarray(Wb.reshape(NL, HB, P, DIN).transpose(2, 1, 0, 3)).astype(np.float16)
    bbT = np.ascontiguousarray(bb.reshape(NL, IB, P).transpose(2, 1, 0))
    bwT = np.ascontiguousarray(bw.reshape(NL, IB, P, DIN).transpose(2, 0, 1, 3)).astype(np.float16)

    in_maps = []
    for k in range(NCORES):
        in_maps.append(
            {
                "xt": xt,
                "ot": otf,
                "oto": np.ascontiguousarray(otf[:, k * CC : (k + 1) * CC]),
                "wc": wcp,
                "bc2": bc2,
                "wws": np.ascontiguousarray(Ww[:, :, k * DSH : (k + 1) * DSH]).astype(np.float16),
                "wbT": wbT,
                "bbT": bbT,
                "bwT": bwT,
            }
        )
    return in_maps


def _run(inputs, trace=False):
    if "nc" not in _nc_cache:
        _nc_cache["nc"] = _build()
    nc = _nc_cache["nc"]
    in_maps = _prep_inputs(**inputs)
    res = run_bass_kernel_spmd(
        nc, in_maps, core_ids=list(range(NCORES)), trace=trace
    )
    parts = []
    for k in range(NCORES):
        ytk = res.results[k]["yt"].astype(np.float32)
        parts.append(ytk.transpose(0, 3, 1, 2).reshape(CC, N, DIN))
    out = np.concatenate(parts, axis=0).reshape(C * N, DIN)
    return out, res


def kernel(**inputs):
    out, _ = _run(inputs, trace=False)
    return out
